# revision 1
# baseline (speedup 1.0000x reference)
"""Trainium2 Bass kernel for margin-ranking + weighted-BCE loss pair.

Math
----
reference:
  margin_loss = sum_{i<j}[ (m - dp*dl) if dp*dl < m else 0 ] / B
              = sum_{i<j} relu(m - prod_ij) / B
  with prod_ij = (p_i - p_j)(l_i - l_j) symmetric in (i,j) and prod_ii = 0:
  S_full := sum_{i,j in [B]^2} relu(m - prod_ij) = 2*S_upper + B*relu(m)
  => margin_loss = S_full/(2B) - relu(m)/2

  M_ij := m - prod_ij = p_i*l_j + l_i*p_j + 1*(m - u_j) + u_i*(-1),  u = p*l
  i.e. a rank-4 outer product -> one matmul materializes any tile of M.

Distribution: the 16x16 grid of 512x512 blocks of M, keeping only the upper
triangle (136 blocks, computed once, off-diag weighted 2x / diag 1x via a
0.5 scale on the diagonal + global 2x folded into the formula). Core c gets
row-bands {c, 15-c} -> always exactly 17 blocks (2 diagonal + 15 off-diag),
so one uniform SPMD program serves all 8 cores; the host feeds each core its
own gathered row/col slices (pure slicing/layout, no arithmetic).

Per block: 4 bf16 matmuls (M=128, N=512) into 4 PSUM banks. The contraction
dim is zero-padded from 4 to 128 so the PE array registers as busy and the
HAM clock gate lifts to 2.4 GHz (K=4 matmuls run at the cold 1.2 GHz clock
forever); a short dummy-matmul stream during setup pre-warms the clock.
Each [128, 2048] PSUM block is consumed by ONE fused relu+accumulate
instruction on ScalarE or VectorE, so the two elementwise engines split the
reduction load. BCE runs on a 1024-element f32 shard per core (exp/ln on
ScalarE, elementwise on the otherwise idle GpSimd). A final ones-matmul
reduces partitions; the host sums the 8 [margin_partial, bce_partial] pairs
and applies closed-form corrections.
"""

import numpy as np
import ml_dtypes

import concourse.bacc as bacc
import concourse.bass as bass
import concourse.mybir as mybir
import concourse.tile as tile
from concourse.bass_utils import run_bass_kernel_spmd

B = 8192
NCORES = 8
SBLK = 512                 # pairwise block side
NBANDS = B // SBLK         # 16
T = 17                     # blocks per core
FL = T * SBLK              # 8704 flattened row/col elements per core
P = 128
P32 = 32
F272 = FL // P32           # 272
BCE_N = B // NCORES        # 1024 -> [128, 8]
BCE_F = BCE_N // P         # 8
NWARM = 8                  # PE clock pre-warm matmuls
NCHUNK = 2 * T             # 34 half-block [128, 1024] relu chunks

# chunks whose relu+reduce runs on ScalarE (rest on VectorE). Chunks 0-3
# are the diagonal blocks and need the 0.5 pre-scale only activation
# provides. ScalarE takes 16 (its chunks cost ~1.37us incl. the
# accumulator read), VectorE 18 (~1.27us each).
ACT_H = frozenset((0, 1, 2, 3)) | frozenset(range(5, 29, 2))

f32 = mybir.dt.float32
bf16 = mybir.dt.bfloat16


def _block_schedule(core: int):
    """17 (row_band, col_band) pairs for `core`; diagonal blocks first."""
    bands = (core, NBANDS - 1 - core)
    blocks = [(bands[0], bands[0]), (bands[1], bands[1])]
    for r in bands:
        for cb in range(r, NBANDS):
            if cb != r:
                blocks.append((r, cb))
    assert len(blocks) == T
    return blocks


def _build_program(margin: float, mode: str = "bf16", skip: tuple = ()):
    from contextlib import ExitStack

    assert mode == "bf16"
    nc = bacc.Bacc("TRN2", target_bir_lowering=False, debug=False,
                   num_devices=NCORES)
    Relu = mybir.ActivationFunctionType.Relu
    Exp = mybir.ActivationFunctionType.Exp
    Ln = mybir.ActivationFunctionType.Ln
    add = mybir.AluOpType.add
    mult = mybir.AluOpType.mult
    amax = mybir.AluOpType.max

    rowp_d = nc.dram_tensor("rowp", [P32, F272], bf16, kind="ExternalInput")
    rowl_d = nc.dram_tensor("rowl", [P32, F272], bf16, kind="ExternalInput")
    colp_d = nc.dram_tensor("colp", [P32, F272], bf16, kind="ExternalInput")
    coll_d = nc.dram_tensor("coll", [P32, F272], bf16, kind="ExternalInput")
    cn_d = nc.dram_tensor("cn", [2, FL], bf16, kind="ExternalInput")
    blg_d = nc.dram_tensor("blg", [P, BCE_F], f32, kind="ExternalInput")
    btg_d = nc.dram_tensor("btg", [P, BCE_F], f32, kind="ExternalInput")
    pw_d = nc.dram_tensor("pw", [P, 1], f32, kind="ExternalInput")
    out_d = nc.dram_tensor("out", [1, 2], f32, kind="ExternalOutput")

    with tile.TileContext(nc) as tc, ExitStack() as ctx:
        big = ctx.enter_context(tc.tile_pool(name="big", bufs=1))
        small = ctx.enter_context(tc.tile_pool(name="small", bufs=1))
        scr = ctx.enter_context(tc.tile_pool(name="scr", bufs=2))
        psum = ctx.enter_context(
            tc.tile_pool(name="psum", bufs=4, space=bass.MemorySpace.PSUM))

        # ---- operand planes ---------------------------------------------
        # partitions 0-3 carry the rank-4 data (lhs: [p_row, l_row, 1,
        # u_row]; rhs: [l_col, p_col, m - u_col, -1]); partitions 4-127 are
        # zeros so K=128 matmuls keep the PE activity monitor warm. Host
        # supplies everything except u and m-u, computed in [32, 272]
        # layout and DMA-gathered into the planes.
        lhs_rep = big.tile([P, FL], bf16, tag="lhs")
        rhs_rep = big.tile([P, FL], bf16, tag="rhs")
        # zero the whole planes first (engines can only start whole-tile at
        # partition 0; a memset is FD-bound so full-tile costs the same as
        # any partition slice); the data rows 0-3 then overwrite.
        nc.vector.memset(lhs_rep[:, :].bitcast(f32), 0.0)
        nc.scalar.memzero(rhs_rep[:, :])

        # PE clock pre-warm: dense K=128 matmuls on a constant tile while
        # the operand planes are still loading.
        wtile = small.tile([P, SBLK], bf16, tag="wtile")
        nc.vector.memset(wtile[:, :], 1.0)
        for i in range(NWARM // 2):
            wpsum = psum.tile([P, 2, SBLK], f32, tag="blk")
            nc.tensor.matmul(wpsum[:, 0, :], wtile[:, 0:P], wtile[:, :],
                             start=True, stop=True)
            nc.tensor.matmul(wpsum[:, 1, :], wtile[:, 0:P], wtile[:, :],
                             start=True, stop=True)

        rp32 = small.tile([P32, F272], bf16, tag="rp32")
        rl32 = small.tile([P32, F272], bf16, tag="rl32")
        cp32 = small.tile([P32, F272], bf16, tag="cp32")
        cl32 = small.tile([P32, F272], bf16, tag="cl32")
        nc.sync.dma_start(out=rp32[:, :], in_=rowp_d[:, :])
        nc.sync.dma_start(out=rl32[:, :], in_=rowl_d[:, :])
        nc.scalar.dma_start(out=cp32[:, :], in_=colp_d[:, :])
        nc.scalar.dma_start(out=cl32[:, :], in_=coll_d[:, :])

        u16 = small.tile([P32, F272], bf16, tag="u16")
        ucol = small.tile([P32, F272], f32, tag="ucol")
        mu16 = small.tile([P32, F272], bf16, tag="mu16")
        nc.gpsimd.tensor_mul(u16[:, :], rp32[:, :], rl32[:, :])
        nc.gpsimd.tensor_mul(ucol[:, :], cp32[:, :], cl32[:, :])
        # mu = -u_col + m  (rounding write into bf16)
        nc.gpsimd.tensor_scalar(mu16[:, :], ucol[:, :], -1.0,
                                float(margin), mult, add)

        nc.sync.dma_start(out=lhs_rep[0:1, :], in_=rowp_d[:, :])
        nc.sync.dma_start(out=lhs_rep[1:2, :], in_=rowl_d[:, :])
        nc.sync.dma_start(out=lhs_rep[2:3, :], in_=cn_d[0:1, :])
        nc.sync.dma_start(out=lhs_rep[3:4, :], in_=u16[:, :])
        nc.scalar.dma_start(out=rhs_rep[0:1, :], in_=coll_d[:, :])
        nc.scalar.dma_start(out=rhs_rep[1:2, :], in_=colp_d[:, :])
        nc.scalar.dma_start(out=rhs_rep[2:3, :], in_=mu16[:, :])
        nc.scalar.dma_start(out=rhs_rep[3:4, :], in_=cn_d[1:2, :])

        # ---- BCE on the 1024-element shard (exp/ln on ScalarE early so
        # its table sets load during setup; elementwise on GpSimd) --------
        zt = small.tile([P, BCE_F], f32, tag="zt")
        tt = small.tile([P, BCE_F], f32, tag="tt")
        pwt = small.tile([P, 1], f32, tag="pwt")
        nc.sync.dma_start(out=zt[:, :], in_=blg_d[:, :])
        nc.sync.dma_start(out=tt[:, :], in_=btg_d[:, :])
        nc.sync.dma_start(out=pwt[:, :], in_=pw_d[:, :])

        mv = small.tile([P, BCE_F], f32, tag="mv")
        zm = small.tile([P, BCE_F], f32, tag="zm")
        e1 = small.tile([P, BCE_F], f32, tag="e1")
        e2 = small.tile([P, BCE_F], f32, tag="e2")
        esum = small.tile([P, BCE_F], f32, tag="esum")
        lg = small.tile([P, BCE_F], f32, tag="lgv")
        so = small.tile([P, BCE_F], f32, tag="so")
        wv = small.tile([P, BCE_F], f32, tag="wv")
        r1 = small.tile([P, BCE_F], f32, tag="r1")
        tz = small.tile([P, BCE_F], f32, tag="tz")
        r2 = small.tile([P, BCE_F], f32, tag="r2")
        pwm1 = small.tile([P, 1], f32, tag="pwm1")
        bce_el = small.tile([P, BCE_F], f32, tag="bce_el")
        bce_acc = small.tile([P, 1], f32, tag="bce_acc")

        if "bce" in skip:
            nc.gpsimd.memset(bce_acc[:, :], 0.0)
        else:
            # mv = relu(-z) = max(-z, 0)
            nc.gpsimd.tensor_scalar_mul(mv[:, :], zt[:, :], -1.0)
            nc.gpsimd.tensor_scalar_max(mv[:, :], mv[:, :], 0.0)
            nc.gpsimd.tensor_add(zm[:, :], zt[:, :], mv[:, :])
            nc.scalar.activation(e1[:, :], mv[:, :], Exp, scale=-1.0)
            nc.scalar.activation(e2[:, :], zm[:, :], Exp, scale=-1.0)
            nc.gpsimd.tensor_add(esum[:, :], e1[:, :], e2[:, :])
            nc.scalar.activation(lg[:, :], esum[:, :], Ln)
            nc.gpsimd.tensor_add(so[:, :], lg[:, :], mv[:, :])
            nc.gpsimd.tensor_scalar_add(pwm1[:, :], pwt[:, :], -1.0)
            nc.gpsimd.tensor_scalar(wv[:, :], tt[:, :], pwm1[:, 0:1], 1.0,
                                    mult, add)
            nc.gpsimd.tensor_mul(r1[:, :], wv[:, :], so[:, :])
            nc.gpsimd.tensor_mul(tz[:, :], tt[:, :], zt[:, :])
            nc.gpsimd.tensor_sub(r2[:, :], zt[:, :], tz[:, :])
            nc.gpsimd.tensor_add(bce_el[:, :], r1[:, :], r2[:, :])
            nc.vector.tensor_reduce(bce_acc[:, :], bce_el[:, :],
                                    axis=mybir.AxisListType.X, op=add)

        # early, dependency-free pieces of the tail
        ones1 = small.tile([P, 1], f32, tag="ones1")
        nc.gpsimd.memset(ones1[:, :], 1.0)

        # ---- the 17 pairwise blocks -------------------------------------
        n_act = len(ACT_H)
        n_dve = NCHUNK - n_act
        acc_a = small.tile([P, n_act], f32, tag="acc_a")
        acc_d = small.tile([P, n_dve], f32, tag="acc_d")

        ia = 0
        idv = 0
        for t in range(T):
            for half in range(2):
                h = 2 * t + half
                pb = psum.tile([P, 2, SBLK], f32, tag="blk")
                for j in range(2):
                    q = 2 * half + j
                    nc.tensor.matmul(
                        pb[:, j, :],
                        lhs_rep[:, SBLK * t + P * q: SBLK * t + P * (q + 1)],
                        rhs_rep[:, SBLK * t: SBLK * (t + 1)],
                        start=True, stop=True,
                    )
                if h in ACT_H:
                    sa = scr.tile([P, 2, SBLK], f32, tag="scr_a")
                    nc.scalar.activation(sa[:, :, :], pb[:, :, :], Relu,
                                         scale=(0.5 if t < 2 else 1.0),
                                         accum_out=acc_a[:, ia: ia + 1])
                    ia += 1
                else:
                    sd = scr.tile([P, 2, SBLK], f32, tag="scr_d")
                    nc.vector.tensor_scalar(sd[:, :, :], pb[:, :, :], 0.0,
                                            0.0, amax, add,
                                            accum_out=acc_d[:, idv: idv + 1])
                    idv += 1
        assert ia == n_act and idv == n_dve

        # ---- final reduction --------------------------------------------
        red_a = small.tile([P, 1], f32, tag="red_a")
        red_d = small.tile([P, 1], f32, tag="red_d")
        stacked = small.tile([P, 2], f32, tag="stacked")
        nc.vector.tensor_reduce(red_a[:, :], acc_a[:, :],
                                axis=mybir.AxisListType.X, op=add)
        nc.vector.tensor_reduce(red_d[:, :], acc_d[:, :],
                                axis=mybir.AxisListType.X, op=add)
        nc.vector.tensor_add(stacked[:, 0:1], red_a[:, :], red_d[:, :])
        nc.vector.tensor_copy(stacked[:, 1:2], bce_acc[:, :])

        if "final" in skip:
            nc.sync.dma_start(out=out_d[:, :], in_=stacked[0:1, 0:2])
        else:
            pfin = psum.tile([1, 2], f32, tag="blk")
            nc.tensor.matmul(pfin[:, :], ones1[:, :], stacked[:, :],
                             start=True, stop=True)
            outt = small.tile([1, 2], f32, tag="outt")
            nc.scalar.copy(outt[:, :], pfin[:, :])
            nc.sync.dma_start(out=out_d[:, :], in_=outt[:, :])

    nc.compile()
    return nc


_programs: dict = {}


def _get_program(margin: float, mode: str = "bf16", skip: tuple = ()):
    key = (margin, mode, skip)
    if key not in _programs:
        _programs[key] = _build_program(margin, mode, skip)
    return _programs[key]


def _make_in_maps(preds, labels, logits, targets, pos_weight, mode="bf16"):
    p = np.ascontiguousarray(np.asarray(preds, np.float32))
    l = np.ascontiguousarray(np.asarray(labels, np.float32))
    z = np.ascontiguousarray(np.asarray(logits, np.float32))
    tg = np.ascontiguousarray(np.asarray(targets, np.float32))
    pw = float(np.asarray(pos_weight, np.float32).reshape(-1)[0])
    ndt = ml_dtypes.bfloat16
    cn = np.empty((2, FL), ndt)
    cn[0, :] = 1.0
    cn[1, :] = -1.0
    in_maps = []
    for c in range(NCORES):
        blocks = _block_schedule(c)
        rowp = np.concatenate([p[SBLK * r: SBLK * (r + 1)] for r, _ in blocks])
        rowl = np.concatenate([l[SBLK * r: SBLK * (r + 1)] for r, _ in blocks])
        colp = np.concatenate([p[SBLK * cb: SBLK * (cb + 1)] for _, cb in blocks])
        coll = np.concatenate([l[SBLK * cb: SBLK * (cb + 1)] for _, cb in blocks])
        in_maps.append({
            "rowp": rowp.astype(ndt).reshape(P32, F272),
            "rowl": rowl.astype(ndt).reshape(P32, F272),
            "colp": colp.astype(ndt).reshape(P32, F272),
            "coll": coll.astype(ndt).reshape(P32, F272),
            "cn": cn,
            "blg": z[BCE_N * c: BCE_N * (c + 1)].reshape(P, BCE_F).copy(),
            "btg": tg[BCE_N * c: BCE_N * (c + 1)].reshape(P, BCE_F).copy(),
            "pw": np.full((P, 1), pw, np.float32),
        })
    return in_maps


def _combine(outs: np.ndarray, margin: float) -> np.ndarray:
    # outs: [NCORES, 1, 2] per-core partials
    s_half = float(outs[:, 0, 0].sum())
    s_bce = float(outs[:, 0, 1].sum())
    margin_loss = s_half / B - max(float(margin), 0.0) / 2.0
    bce_loss = s_bce / B
    return np.array([margin_loss, bce_loss], dtype=np.float32)


MODE = "bf16"


def _run(inputs: dict, trace: bool = False, mode: str | None = None,
         **spmd_kwargs):
    if mode is None:
        mode = MODE
    m = float(np.asarray(inputs["margin"]))
    nc = _get_program(m, mode)
    in_maps = _make_in_maps(inputs["preds"], inputs["labels"],
                            inputs["logits"], inputs["targets"],
                            inputs["pos_weight"], mode=mode)
    res = run_bass_kernel_spmd(nc, in_maps, core_ids=list(range(NCORES)),
                               trace=trace, **spmd_kwargs)
    outs = np.stack([np.asarray(r["out"], np.float32) for r in res.results])
    return _combine(outs, m), res


def kernel(preds, labels, logits, targets, pos_weight, margin):
    out, _ = _run(dict(preds=preds, labels=labels, logits=logits,
                       targets=targets, pos_weight=pos_weight,
                       margin=margin))
    return out

